# revision 5
# baseline (speedup 1.0000x reference)
"""Trainium2 Bass kernel for:
    logits4 = einsum('bic,bjc->bijc', Q, K) + bias      # [B,I,J,C]
    output  = sigmoid(logits4).mean(axis=-2)            # [B,I,C]
    attention_logits = einsum('bic,bjc->bij', Q, K)     # [B,I,J]
    return (output, attention_logits)

B,I,J,C = 4,512,512,512. Runs SPMD on 8 NeuronCores: core k handles
(b = k//2, h = k%2) with the sigmoid-mean part sharded over C-halves
(all I), and the attention-logits matmul sharded over I-halves (all C).

Per-core dataflow (ScalarE-bound; ACT busy ~= 2048 elems/c * 0.83ns):
  - TensorE builds the biased outer-product tiles
    P[i,j] = Q[i,c]*K[j,c] + bias[c] with contraction-dim-2 matmuls:
    lhsT = (Q^T row c, ones) x i-block, rhs = (K^T row c, bias[c]*ones).
    Operand pairs live on SBUF partitions 0-1, packed c-major along the
    free dim and streamed from DRAM in 8-c chunks.
  - PSUM groups of GROUP_BANKS banks (2 => [128,1024], finer PE->ACT
    pipelining); ScalarE sigmoid reads ACT_N elems per instruction
    (1024 = 2 banks is HW-legal; flat 2048 hangs the engine).
  - VectorE reduces over j: pairwise TT adds (2x bf16) then one 1x
    tensor_reduce; accumulated means land in `stage`.
  - attention_logits: QK^T matmuls issued mid-loop (chunk LOGITS_AT) so
    PE/DVE/DMA for them hide under ACT; only the mean DMA-out tails.
"""
import os

if "JAX_PLATFORMS" in os.environ and "axon" not in os.environ["JAX_PLATFORMS"]:
    # the bass kernel executes through the axon PJRT backend
    os.environ["JAX_PLATFORMS"] = ""

import numpy as np
import ml_dtypes

import concourse.bacc as bacc
import concourse.mybir as mybir
from concourse import tile
from concourse.bass_utils import run_bass_kernel_spmd

B, I, J, C = 4, 512, 512, 512
NCORES = 8
CH = C // 2          # c-half per core
IH = I // 2          # i-half per core
NIB = I // 128       # i-blocks (4)
CHUNK = 8            # c's per staged operand tile

BF16 = mybir.dt.bfloat16
F32 = mybir.dt.float32
ADD = mybir.AluOpType.add

ACT_N = 1024          # free elems per ACTIVATE (1024 HW-verified; 2048 hangs flat)
GROUP_BANKS = 2       # PSUM banks per matmul/ACT group (2 or 4)
DVE_ADDS = 2          # pairwise TT halvings before the 1x tensor_reduce
LOGITS_AT = 3         # chunk index at which to issue the attention-logits work
PASSES = 1            # repeat the main loop (timing experiments only)


def build_nc():
    nc = bacc.Bacc("TRN2", target_bir_lowering=False, debug=False, num_devices=NCORES)

    # qp: [0, c*I + i] = Q^T[c, i], [1, :] = 1.0
    # kp: [0, c*J + j] = K^T[c, j], [1, c*J + j] = bias[c]
    qp = nc.dram_tensor("qp", [2, CH * I], BF16, kind="ExternalInput")
    kp = nc.dram_tensor("kp", [2, CH * J], BF16, kind="ExternalInput")
    qt = nc.dram_tensor("qt", [C, IH], BF16, kind="ExternalInput")   # Q^T, i-half
    kt = nc.dram_tensor("kt", [C, J], BF16, kind="ExternalInput")    # K^T, full
    out_mean = nc.dram_tensor("out_mean", [I, CH], F32, kind="ExternalOutput")
    out_logits = nc.dram_tensor("out_logits", [IH, J], F32, kind="ExternalOutput")

    GB = GROUP_BANKS
    GN = GB * 512            # free elems per PSUM group
    NG = NIB * J // GN       # groups per c
    # GB=2: 3 bufs (6 banks) + dedicated 2-bank logits tile.
    # GB=4: 2 bufs = all 8 banks; logits reuses an mp rotation slot.
    MP_BUFS = 3 if GB == 2 else 2

    with tile.TileContext(nc) as tc:
        with (
            tc.tile_pool(name="sb", bufs=1) as sb,
            tc.tile_pool(name="st", bufs=3) as st,
            tc.tile_pool(name="mp", bufs=MP_BUFS, space="PSUM") as mp,
            tc.tile_pool(name="lp", bufs=1, space="PSUM") as lp,
            tc.tile_pool(name="sg", bufs=3) as sg,
        ):
            # main-loop chunk 0/1 operands first so PE can start immediately
            pre_qs, pre_ks = [], []
            for chunk in range(2):
                c0 = chunk * CHUNK
                qs = st.tile([2, CHUNK * I], BF16, tag="qs")
                nc.sync.dma_start(qs[:], qp[:, c0 * I : (c0 + CHUNK) * I])
                ks = st.tile([2, CHUNK * J], BF16, tag="ks")
                nc.sync.dma_start(ks[:], kp[:, c0 * J : (c0 + CHUNK) * J])
                pre_qs.append(qs)
                pre_ks.append(ks)

            qt_t = []
            kt_t = []
            for t in range(C // 128):
                a = sb.tile([128, IH], BF16, tag=f"qt{t}")
                nc.sync.dma_start(a[:], qt[128 * t : 128 * (t + 1), :])
                qt_t.append(a)
                b = sb.tile([128, J], BF16, tag=f"kt{t}")
                nc.sync.dma_start(b[:], kt[128 * t : 128 * (t + 1), :])
                kt_t.append(b)

            # means land here: stage[p, ib*CH + cc] = mean[ib*128+p, cc]
            stage = sb.tile([128, NIB * CH], F32, tag="stage")

            def do_logits():
                # GB=2: dedicated 2-bank tile; GB=4: reuse an mp "ps" slot
                # (same tag+shape so the pool doesn't grow past 8 banks).
                if GB == 2:
                    ps_lg = lp.tile([128, 2 * J], F32, tag="lg")
                else:
                    ps_lg = mp.tile([128, GN], F32, tag="ps")
                for it in range(IH // 128):
                    for cb in range(C // 128):
                        nc.tensor.matmul(
                            ps_lg[:, it * J : (it + 1) * J],
                            qt_t[cb][:, it * 128 : (it + 1) * 128],
                            kt_t[cb][:],
                            start=(cb == 0),
                            stop=(cb == C // 128 - 1),
                        )
                for it in range(IH // 128):
                    lg = sb.tile([128, J], F32, tag=f"lg{it}")
                    nc.vector.tensor_copy(lg[:], ps_lg[:, it * J : (it + 1) * J])
                    nc.sync.dma_start(out_logits[it * 128 : (it + 1) * 128, :], lg[:])

            for _ in range(PASSES):
              for chunk in range(CH // CHUNK):
                c0 = chunk * CHUNK
                if chunk < 2:
                    qs, ks = pre_qs[chunk], pre_ks[chunk]
                else:
                    qs = st.tile([2, CHUNK * I], BF16, tag="qs")
                    nc.sync.dma_start(qs[:], qp[:, c0 * I : (c0 + CHUNK) * I])
                    ks = st.tile([2, CHUNK * J], BF16, tag="ks")
                    nc.sync.dma_start(ks[:], kp[:, c0 * J : (c0 + CHUNK) * J])
                if chunk == LOGITS_AT:
                    do_logits()
                for m in range(CHUNK):
                    c = c0 + m
                    sigw = sg.tile([128, NIB * J], BF16, tag="sigw")
                    for g in range(NG):
                        ps = mp.tile([128, GN], F32, tag="ps")
                        for ib in range(GB):
                            nc.tensor.matmul(
                                ps[:, ib * J : (ib + 1) * J],
                                qs[
                                    :,
                                    m * I
                                    + (g * GB + ib) * 128 : m * I
                                    + (g * GB + ib + 1) * 128,
                                ],
                                ks[:, m * J : (m + 1) * J],
                                start=True,
                                stop=True,
                            )
                        for a0 in range(0, GN, ACT_N):
                            nc.scalar.activation(
                                sigw[:, g * GN + a0 : g * GN + a0 + ACT_N],
                                ps[:, a0 : a0 + ACT_N],
                                mybir.ActivationFunctionType.Sigmoid,
                            )
                    # DVE: pairwise adds at 2x then one 1x reduce per c
                    cur = sigw[:].rearrange("p (ib j) -> p ib j", ib=NIB)
                    w = J
                    for _a in range(DVE_ADDS):
                        t1 = sg.tile([128, NIB * (w // 2)], BF16, tag=f"t{_a}")
                        t13 = t1[:].rearrange("p (ib j) -> p ib j", ib=NIB)
                        nc.vector.tensor_add(t13, cur[:, :, : w // 2], cur[:, :, w // 2 :])
                        cur = t13
                        w //= 2
                    nc.vector.tensor_reduce(
                        stage[:].rearrange("p (ib cc) -> p ib cc", ib=NIB)[
                            :, :, c : c + 1
                        ],
                        cur,
                        axis=mybir.AxisListType.X,
                        op=ADD,
                    )

            nc.vector.tensor_scalar_mul(stage[:], stage[:], 1.0 / J)
            for ib in range(NIB):
                nc.sync.dma_start(
                    out_mean[ib * 128 : (ib + 1) * 128, :],
                    stage[:, ib * CH : (ib + 1) * CH],
                )

    nc.compile()
    return nc


def make_in_maps(Q, K, bias):
    Q = np.asarray(Q, dtype=np.float32)
    K = np.asarray(K, dtype=np.float32)
    bias = np.asarray(bias, dtype=np.float32)
    in_maps = []
    for core in range(NCORES):
        b, h = core // 2, core % 2
        cs = slice(h * CH, (h + 1) * CH)
        QT = Q[b].T.astype(ml_dtypes.bfloat16)  # [C, I]
        KT = K[b].T.astype(ml_dtypes.bfloat16)  # [C, J]
        qp = np.empty((2, CH, I), dtype=ml_dtypes.bfloat16)
        qp[0] = QT[cs]
        qp[1] = np.float32(1.0)
        kp = np.empty((2, CH, J), dtype=ml_dtypes.bfloat16)
        kp[0] = KT[cs]
        kp[1] = bias[cs].astype(ml_dtypes.bfloat16)[:, None]
        in_maps.append(
            {
                "qp": qp.reshape(2, CH * I),
                "kp": kp.reshape(2, CH * J),
                "qt": np.ascontiguousarray(QT[:, h * IH : (h + 1) * IH]),
                "kt": np.ascontiguousarray(KT),
            }
        )
    return in_maps


def assemble(results):
    output = np.empty((B, I, C), dtype=np.float32)
    attention_logits = np.empty((B, I, J), dtype=np.float32)
    for core in range(NCORES):
        b, h = core // 2, core % 2
        output[b, :, h * CH : (h + 1) * CH] = results[core]["out_mean"]
        attention_logits[b, h * IH : (h + 1) * IH, :] = results[core]["out_logits"]
    return output, attention_logits


def build_null_nc():
    """Minimal kernel used by test.py to measure dispatch overhead."""
    nc = bacc.Bacc("TRN2", target_bir_lowering=False, debug=False, num_devices=NCORES)
    x = nc.dram_tensor("x", [8, 8], F32, kind="ExternalInput")
    y = nc.dram_tensor("y", [8, 8], F32, kind="ExternalOutput")
    with tile.TileContext(nc) as tc:
        with tc.tile_pool(name="p", bufs=1) as pool:
            t = pool.tile([8, 8], F32)
            nc.sync.dma_start(t[:], x[:])
            nc.sync.dma_start(y[:], t[:])
    nc.compile()
    return nc


_NC = None


def get_nc():
    global _NC
    if _NC is None:
        _NC = build_nc()
    return _NC


def run(Q, K, bias, **kwargs):
    nc = get_nc()
    res = run_bass_kernel_spmd(
        nc, make_in_maps(Q, K, bias), core_ids=list(range(NCORES)), **kwargs
    )
    return res


def kernel(Q, K, bias):
    res = run(Q, K, bias)
    return assemble(res.results)
